# revision 13
# baseline (speedup 1.0000x reference)
"""Trainium2 Bass kernel for a 3-layer GCN (+linear head, softmax).

Contract: kernel(**inputs) takes FULL unsharded inputs (as produced by the
problem's setup_inputs()) and returns the FULL [50000, 10] float32 output.

Strategy (8 NeuronCores, graph/data parallel):
  - dst-node ranges sharded across cores (6250 nodes/core); small weight
    matrices replicated; self-loops appended to the edge list.
  - per GCN layer: each core computes z = h @ W for its own nodes (PE, f32),
    casts to bf16 and AllGathers the full 50k-row feature table into HBM.
  - per-edge source rows are fetched with gpsimd.dma_gather (256B rows);
    aggregation is a PE matmul against a weighted one-hot built on DVE with a
    single dual-op tensor_scalar (is_equal then mult); PSUM accumulates in f32.
  - all GCN normalization (dinv[src]*ew*dinv[dst]) is folded into the one-hot
    weights on the host (static graph preprocessing), so the device epilogue is
    just +bias and ReLU.
  - final linear head + softmax computed per node tile on device.

dma_gather's indices are int16, so the 50176-row table is split into two row
segments; each (window, segment) edge group is padded to 128-edge chunks and
the two segments are processed in two passes (pass-0 partial sums parked in
SBUF).
"""

import sys

sys.path.insert(0, "/opt/trn_rl_repo")

from contextlib import ExitStack

import numpy as np

import concourse.bass as bass
import concourse.tile as tile
from concourse import bacc, mybir
from concourse.bass_utils import run_bass_kernel_spmd


# ---------------------------------------------------------------- problem cfg
class CFG:
    N = 50000
    E = 800000
    F = 256
    H = 128
    C = 10
    NCORES = 8
    SEG = 32768          # int16 row-index limit for dma_gather
    WINW = 128           # dst window width (one-hot width, PSUM agg columns)
    BATCH = 32           # chunks (of 128 edges) per dma_gather call
    SINGLE_PACKET = False  # single_packet caps at 1024 idxs (64 desc/engine)

    def __init__(self, **kw):
        for k, v in kw.items():
            setattr(self, k, v)
        assert self.N % self.NCORES == 0
        self.NC_NODES = self.N // self.NCORES            # nodes per core
        self.NTILES = (self.NC_NODES + 127) // 128       # 128-node tiles/core
        self.NC_PAD = self.NTILES * 128                  # padded nodes/core
        self.TROWS = self.NCORES * self.NC_PAD           # table rows (padded)
        self.N_WIN = (self.NC_NODES + self.WINW - 1) // self.WINW
        assert self.WINW == 128 and self.H == 128


# ---------------------------------------------------------------- host planner
def _table_row(cfg, node):
    """Global node id -> row in the AllGathered table.

    Table layout: rank-major, then partition-major within a core's [128, NTILES]
    z staging tile: row = (c*128 + p)*NTILES + t  with node = c*NC + t*128 + p."""
    c = node // cfg.NC_NODES
    local = node % cfg.NC_NODES
    t = local // 128
    p = local % 128
    return (c * 128 + p) * cfg.NTILES + t


def build_plan(cfg, edge_index, edge_weight):
    """Shared schedule + per-core staged arrays."""
    src = np.asarray(edge_index[0], np.int64)
    dst = np.asarray(edge_index[1], np.int64)
    ew = np.asarray(edge_weight, np.float32)

    loop = np.arange(cfg.N, dtype=np.int64)
    src = np.concatenate([src, loop])
    dst = np.concatenate([dst, loop])
    ew = np.concatenate([ew, np.ones(cfg.N, np.float32)])

    deg = np.zeros(cfg.N, np.float32)
    np.add.at(deg, dst, ew)
    dinv = (1.0 / np.sqrt(deg)).astype(np.float32)
    wnorm = (ew * dinv[src] * dinv[dst]).astype(np.float32)

    row = _table_row(cfg, src)
    seg = row // cfg.SEG
    n_seg = int(seg.max()) + 1
    assert n_seg <= 2 and cfg.TROWS - cfg.SEG < cfg.SEG
    n_seg = 2 if cfg.TROWS > cfg.SEG else 1

    core = dst // cfg.NC_NODES
    win = (dst % cfg.NC_NODES) // cfg.WINW

    order = np.lexsort((row, win, seg, core))
    src, dst, wnorm, core, win, seg, row = (
        a[order] for a in (src, dst, wnorm, core, win, seg, row))
    key = (core * n_seg + seg) * cfg.N_WIN + win
    starts = np.concatenate([[0], np.flatnonzero(np.diff(key)) + 1])
    ends = np.concatenate([starts[1:], [len(key)]])
    groups = {int(key[a]): (int(a), int(b)) for a, b in zip(starts, ends)}

    # shared chunk counts per (seg, win): max over cores, >= 1
    chunks_sw = np.zeros((n_seg, cfg.N_WIN), np.int64)
    for s in range(n_seg):
        for w in range(cfg.N_WIN):
            mx = max(
                groups.get((c * n_seg + s) * cfg.N_WIN + w, (0, 0))[1]
                - groups.get((c * n_seg + s) * cfg.N_WIN + w, (0, 0))[0]
                for c in range(cfg.NCORES))
            chunks_sw[s, w] = max(1, -(-mx // 128))
    total_chunks = int(chunks_sw.sum())

    # shared schedule: per chunk (seg, win, start, stop); batch spans per seg
    sched = []
    for s in range(n_seg):
        for w in range(cfg.N_WIN):
            for j in range(chunks_sw[s, w]):
                sched.append((s, w, j == 0, j == int(chunks_sw[s, w]) - 1))
    batches = []  # (chunk_lo, chunk_hi)
    pos = 0
    for s in range(n_seg):
        nch = int(chunks_sw[s].sum())
        for lo in range(0, nch, cfg.BATCH):
            batches.append((pos + lo, pos + min(lo + cfg.BATCH, nch)))
        pos += nch

    # per-core staged edge arrays
    per_core = []
    for c in range(cfg.NCORES):
        idx16 = np.zeros((total_chunks, 128), np.int16)
        drel = np.zeros((total_chunks, 128), np.float32)
        wn = np.zeros((total_chunks, 128), np.float32)
        pos = 0
        for s in range(n_seg):
            for w in range(cfg.N_WIN):
                k = (c * n_seg + s) * cfg.N_WIN + w
                nch = int(chunks_sw[s, w])
                if k in groups:
                    a, b = groups[k]
                    e_row = row[a:b] - s * cfg.SEG
                    e_rel = (dst[a:b] % cfg.NC_NODES) % cfg.WINW
                    e_wn = wnorm[a:b]
                else:
                    e_row = e_rel = np.zeros(0, np.int64)
                    e_wn = np.zeros(0, np.float32)
                pad = nch * 128 - len(e_row)
                idx16[pos:pos + nch] = np.concatenate(
                    [e_row, np.zeros(pad, np.int64)]).reshape(nch, 128)
                drel[pos:pos + nch] = np.concatenate(
                    [e_rel, np.zeros(pad, np.int64)]).reshape(nch, 128)
                wn[pos:pos + nch] = np.concatenate(
                    [e_wn, np.zeros(pad, np.float32)]).reshape(nch, 128)
                pos += nch

        # wrap indices for dma_gather: per batch, [p, s16] = idx[s16*16 + p%16]
        idx_wrap = np.zeros((128, total_chunks * 8), np.int16)
        for lo, hi in batches:
            flat = idx16[lo:hi].reshape(-1)            # (hi-lo)*128
            wrapped = flat.reshape(-1, 16).T           # [16, (hi-lo)*8]
            idx_wrap[:, lo * 8:hi * 8] = np.tile(wrapped, (8, 1))

        per_core.append(dict(
            idx_wrap=idx_wrap,
            dstrel=np.ascontiguousarray(drel.T),       # [128, total_chunks]
            wnorm=np.ascontiguousarray(wn.T),          # [128, total_chunks]
        ))

    plan = dict(n_seg=n_seg, chunks_sw=chunks_sw, total_chunks=total_chunks,
                sched=sched, batches=batches, dinv=dinv)
    return plan, per_core


def stage_core_inputs(cfg, plan, per_core, x, Ws, bs, fc_w, fc_b):
    """Build per-core in_maps for run_bass_kernel_spmd."""
    F, H, C = cfg.F, cfg.H, cfg.C
    KF = F // 128
    iota = np.tile(np.arange(cfg.WINW, dtype=np.float32), (128, 1))
    w1 = np.ascontiguousarray(
        np.asarray(Ws[0], np.float32).reshape(KF, 128, H))
    in_maps = []
    for c in range(cfg.NCORES):
        lo = c * cfg.NC_NODES
        xcol = np.zeros((F, cfg.NC_PAD), np.float32)
        xcol[:, :cfg.NC_NODES] = np.asarray(
            x[lo:lo + cfg.NC_NODES], np.float32).T
        xs = np.ascontiguousarray(xcol.reshape(KF, 128, cfg.NC_PAD))
        pc = per_core[c]
        in_maps.append({
            "x_in": xs,
            "idx_in": pc["idx_wrap"],
            "dstrel_in": pc["dstrel"],
            "wnorm_in": pc["wnorm"],
            "iota_in": iota,
            "w1_in": w1,
            "w2_in": np.asarray(Ws[1], np.float32),
            "w3_in": np.asarray(Ws[2], np.float32),
            "b1_in": np.asarray(bs[0], np.float32).reshape(H, 1),
            "b2_in": np.asarray(bs[1], np.float32).reshape(H, 1),
            "b3_in": np.asarray(bs[2], np.float32).reshape(H, 1),
            "fcw_in": np.asarray(fc_w, np.float32),
            "fcb_in": np.tile(np.asarray(fc_b, np.float32), (128, 1)),
        })
    return in_maps


# ---------------------------------------------------------------- device build
def build_nc(cfg, plan):
    f32 = mybir.dt.float32
    bf16 = mybir.dt.bfloat16
    i16 = mybir.dt.int16
    NT, NC_PAD, WINW, H, C = cfg.NTILES, cfg.NC_PAD, cfg.WINW, cfg.H, cfg.C
    KF = cfg.F // 128
    TC = plan["total_chunks"]
    n_seg = plan["n_seg"]
    sched, batches = plan["sched"], plan["batches"]

    nc = bacc.Bacc("TRN2", target_bir_lowering=False, debug=False,
                   num_devices=cfg.NCORES)
    dt_in = {}
    for name, shape, dt in [
        ("x_in", [KF, 128, NC_PAD], f32),
        ("idx_in", [128, TC * 8], i16),
        ("dstrel_in", [128, TC], f32),
        ("wnorm_in", [128, TC], f32),
        ("iota_in", [128, WINW], f32),
        ("w1_in", [KF, 128, H], f32),
        ("w2_in", [H, H], f32),
        ("w3_in", [H, H], f32),
        ("b1_in", [H, 1], f32),
        ("b2_in", [H, 1], f32),
        ("b3_in", [H, 1], f32),
        ("fcw_in", [H, C], f32),
        ("fcb_in", [128, C], f32),
    ]:
        dt_in[name] = nc.dram_tensor(name, shape, dt, kind="ExternalInput").ap()
    y_ap = nc.dram_tensor("y", [128, NT * C], f32, kind="ExternalOutput").ap()

    with tile.TileContext(nc) as tc, ExitStack() as ctx:
        P = lambda name, bufs=1, **kw: ctx.enter_context(
            tc.tile_pool(name=name, bufs=bufs, **kw))
        const_p = P("const")
        hp = P("h", bufs=2)
        zp = P("zsb", bufs=1)
        partial_p = P("partial")
        msgs_p = P("msgs", bufs=2)
        oh_p = P("onehot", bufs=6)
        psum_z = P("psum_z", bufs=2, space="PSUM")
        psum_agg = P("psum_agg", bufs=2, space="PSUM")
        psum_fc = P("psum_fc", bufs=2, space="PSUM")
        dram_p = P("dram", bufs=1, space="DRAM")
        sm_p = P("softmax", bufs=2)

        # ---- load constants / edge metadata into SBUF (once)
        def load(name, shape2d, dt, src_ap=None):
            t = const_p.tile(shape2d, dt, name=f"sb_{name}")
            nc.sync.dma_start(t[:], (src_ap if src_ap is not None
                                     else dt_in[name])[:])
            return t

        x_ctx = ExitStack()
        x_p = x_ctx.enter_context(tc.tile_pool(name="xpool", bufs=1))
        x_sb = x_p.tile([128, KF * NC_PAD], f32, name="x_sb")
        nc.sync.dma_start(
            x_sb[:].rearrange("p (k n) -> p k n", k=KF),
            dt_in["x_in"].rearrange("k p n -> p k n"))
        idx_sb = load("idx_in", [128, TC * 8], i16)
        drel_sb = load("dstrel_in", [128, TC], f32)
        wn_sb = load("wnorm_in", [128, TC], f32)
        iota_f = load("iota_in", [128, WINW], f32)
        iota_sb = const_p.tile([128, WINW], bf16, name="iota_bf")
        nc.vector.tensor_copy(iota_sb[:], iota_f[:])
        w1_sb = const_p.tile([128, KF * H], f32, name="w1_sb")
        nc.sync.dma_start(w1_sb[:].rearrange("p (k h) -> p k h", k=KF),
                          dt_in["w1_in"].rearrange("k p h -> p k h"))
        w2_sb = load("w2_in", [H, H], f32)
        w3_sb = load("w3_in", [H, H], f32)
        b_sb = [load(n, [H, 1], f32) for n in ("b1_in", "b2_in", "b3_in")]
        fcw_sb = load("fcw_in", [H, C], f32)
        fcb_sb = load("fcb_in", [128, C], f32)

        partial = partial_p.tile([128, cfg.N_WIN * WINW], f32, name="partial")

        ag_ins = [dram_p.tile([128, NT * H], bf16, name=f"ag_in{i}",
                              tag=f"ag_in{i}") for i in range(3)]
        ag_outs = [dram_p.tile([cfg.NCORES * 128, NT * H], bf16,
                               name=f"ag_out{i}", tag=f"ag_out{i}",
                               addr_space="Shared") for i in range(3)]

        h_cur = x_sb  # layer-1 input, [128, KF*NC_PAD]
        k_cur = KF
        for li in range(3):
            w_sb = (w1_sb, w2_sb, w3_sb)[li]
            # ---- dense: z[t] = h(:,t-slice).T @ W  (node-major psum), cast bf16
            z_sb = zp.tile([128, NT * H], bf16, name=f"z{li}", tag="z")
            for t in range(NT):
                pz = psum_z.tile([128, H], f32, tag="pz")
                for k in range(k_cur):
                    nc.tensor.matmul(
                        pz[:],
                        h_cur[:, k * NC_PAD + t * 128:
                              k * NC_PAD + (t + 1) * 128],
                        w_sb[:, k * H:(k + 1) * H] if k_cur > 1
                        else w_sb[:, :],
                        start=(k == 0), stop=(k == k_cur - 1))
                nc.vector.tensor_copy(z_sb[:, t * H:(t + 1) * H], pz[:])
            if li == 0:
                x_ctx.close()  # free x_sb SBUF after layer-1 transform
            ag_in, ag_out = ag_ins[li], ag_outs[li]
            # gather-table view: row r=(c*128+p)*NT+t -> 128 bf16 els
            table = ag_out[:].rearrange("a (t h) -> (a t) h", h=H)
            nc.sync.dma_start(ag_in[:], z_sb[:])
            nc.gpsimd.collective_compute(
                "AllGather", mybir.AluOpType.bypass,
                replica_groups=[list(range(cfg.NCORES))],
                ins=[ag_in.opt()], outs=[ag_out.opt()])

            # ---- gather batches
            msgs_tiles = {}
            for bi, (lo, hi) in enumerate(batches):
                bc = hi - lo
                m = msgs_p.tile([128, cfg.BATCH, H], bf16, tag="msgs",
                                name=f"m{li}_{bi}")
                seg = sched[lo][0]
                seg_lo = seg * cfg.SEG
                seg_rows = min(cfg.SEG, cfg.TROWS - seg_lo)
                nc.gpsimd.dma_gather(
                    m[:, :bc, :],
                    table[seg_lo:seg_lo + seg_rows, :],
                    idx_sb[:, lo * 8:hi * 8],
                    bc * 128, bc * 128, H,
                    single_packet=cfg.SINGLE_PACKET)
                msgs_tiles[lo] = m

            # ---- scatter: weighted one-hot matmul, accumulate per window
            h_next = hp.tile([128, NC_PAD], f32, name=f"h{li + 1}", tag="h")
            pa = None
            for ci, (s, w, st, sp) in enumerate(sched):
                blo = max(l for (l, h_) in batches if l <= ci)
                m = msgs_tiles[blo]
                oh = oh_p.tile([128, WINW], bf16, tag="oh")
                nc.vector.tensor_scalar(
                    oh[:], iota_sb[:],
                    drel_sb[:, ci:ci + 1], wn_sb[:, ci:ci + 1],
                    mybir.AluOpType.is_equal, mybir.AluOpType.mult)
                if st:
                    pa = psum_agg.tile([128, WINW], f32, tag="pagg")
                nc.tensor.matmul(pa[:], m[:, ci - blo, :], oh[:],
                                 start=st, stop=sp)
                if sp:
                    wsl = slice(w * WINW, (w + 1) * WINW)
                    if s == 0 and n_seg == 2:
                        nc.vector.tensor_copy(partial[:, wsl], pa[:])
                    else:
                        if n_seg == 2:
                            nc.vector.tensor_tensor(
                                h_next[:, wsl], pa[:], partial[:, wsl],
                                mybir.AluOpType.add)
                        else:
                            nc.vector.tensor_copy(h_next[:, wsl], pa[:])
                        nc.vector.tensor_scalar(
                            h_next[:, wsl], h_next[:, wsl],
                            b_sb[li][:], 0.0,
                            mybir.AluOpType.add, mybir.AluOpType.max)
            h_cur = h_next
            k_cur = 1

        # ---- fc head + softmax (node-major tiles)
        logit = sm_p.tile([128, NT, C], f32, name="logit")
        for t in range(NT):
            pf = psum_fc.tile([128, C], f32, tag="pfc")
            nc.tensor.matmul(pf[:], h_cur[:, t * 128:(t + 1) * 128],
                             fcw_sb[:], start=True, stop=True)
            nc.vector.tensor_tensor(logit[:, t, :], pf[:], fcb_sb[:],
                                    mybir.AluOpType.add)
        rmax = sm_p.tile([128, NT], f32, name="rmax")
        nc.vector.tensor_reduce(rmax[:], logit[:], mybir.AxisListType.X,
                                mybir.AluOpType.max)
        shifted = sm_p.tile([128, NT, C], f32, name="shifted")
        for t in range(NT):
            nc.vector.tensor_scalar(
                shifted[:, t, :], logit[:, t, :], rmax[:, t:t + 1], None,
                mybir.AluOpType.subtract)
        expd = sm_p.tile([128, NT, C], f32, name="expd")
        nc.scalar.activation(expd[:], shifted[:],
                             mybir.ActivationFunctionType.Exp)
        esum = sm_p.tile([128, NT], f32, name="esum")
        nc.vector.tensor_reduce(esum[:], expd[:], mybir.AxisListType.X,
                                mybir.AluOpType.add)
        rinv = sm_p.tile([128, NT], f32, name="rinv")
        nc.vector.reciprocal(rinv[:], esum[:])
        prob = sm_p.tile([128, NT, C], f32, name="prob")
        for t in range(NT):
            nc.vector.tensor_scalar(
                prob[:, t, :], expd[:, t, :], rinv[:, t:t + 1], None,
                mybir.AluOpType.mult)
        nc.sync.dma_start(y_ap[:], prob[:].rearrange("p t c -> p (t c)"))

    nc.compile()
    return nc


# ---------------------------------------------------------------- entry point
_CACHE = {}


def _get_built(cfg_key, cfg, edge_index, edge_weight):
    key = (cfg_key, hash(edge_index.tobytes()) ^ hash(edge_weight.tobytes()))
    if key not in _CACHE:
        plan, per_core = build_plan(cfg, edge_index, edge_weight)
        nc = build_nc(cfg, plan)
        _CACHE.clear()
        _CACHE[key] = (plan, per_core, nc)
    return _CACHE[key]


def kernel(x, edge_index, edge_weight, W1, b1, W2, b2, W3, b3, fc_w, fc_b,
           **extra):
    cfg = CFG()
    x = np.asarray(x, np.float32)
    edge_index = np.asarray(edge_index)
    edge_weight = np.asarray(edge_weight, np.float32)
    plan, per_core, nc = _get_built("default", cfg, edge_index, edge_weight)
    in_maps = stage_core_inputs(
        cfg, plan, per_core, x, [W1, W2, W3], [b1, b2, b3], fc_w, fc_b)
    res = run_bass_kernel_spmd(nc, in_maps, core_ids=list(range(cfg.NCORES)))
    out = np.zeros((cfg.N, cfg.C), np.float32)
    for c in range(cfg.NCORES):
        yc = res.results[c]["y"].reshape(128, cfg.NTILES, cfg.C)
        yc = yc.transpose(1, 0, 2).reshape(cfg.NC_PAD, cfg.C)
        out[c * cfg.NC_NODES:(c + 1) * cfg.NC_NODES] = yc[:cfg.NC_NODES]
    return out


# revision 19
# speedup vs baseline: 3688.4306x; 3688.4306x over previous
"""Trainium2 Bass kernel for a 3-layer GCN (+linear head, softmax).

Contract: kernel(**inputs) takes FULL unsharded inputs (as produced by the
problem's setup_inputs()) and returns the FULL [50000, 10] float32 output.

Strategy (8 NeuronCores, graph/data parallel):
  - dst-node ranges sharded across cores (6250 nodes/core); small weight
    matrices replicated; self-loops appended to the edge list.
  - per GCN layer: each core computes z = h @ W for its own nodes (PE, f32),
    casts to bf16 and AllGathers the full 50k-row feature table into HBM.
  - per-edge source rows are fetched with gpsimd.dma_gather (256B rows);
    aggregation is a PE matmul against a weighted one-hot built on DVE with a
    single dual-op tensor_scalar (is_equal then mult); PSUM accumulates in f32.
  - all GCN normalization (dinv[src]*ew*dinv[dst]) is folded into the one-hot
    weights on the host (static graph preprocessing), so the device epilogue is
    just +bias and ReLU.
  - final linear head + softmax computed per node tile on device.

dma_gather's indices are int16, so the 50176-row table is split into two row
segments; each (window, segment) edge group is padded to 128-edge chunks and
the two segments are processed in two passes (pass-0 partial sums parked in
SBUF).
"""

import sys

sys.path.insert(0, "/opt/trn_rl_repo")

from contextlib import ExitStack

import numpy as np

import concourse.bass as bass
import concourse.tile as tile
from concourse import bacc, mybir
from concourse.bass_utils import run_bass_kernel_spmd


# ---------------------------------------------------------------- problem cfg
class CFG:
    N = 50000
    E = 800000
    F = 256
    H = 128
    C = 10
    NCORES = 8
    SEG = 32768          # int16 row-index limit for dma_gather
    WINW = 128           # dst window width (one-hot width, PSUM agg columns)
    BATCH = 32           # chunks (of 128 edges) per dma_gather call
    SINGLE_PACKET = False  # single_packet caps at 1024 idxs (64 desc/engine)

    def __init__(self, **kw):
        for k, v in kw.items():
            setattr(self, k, v)
        assert self.N % self.NCORES == 0
        self.NC_NODES = self.N // self.NCORES            # nodes per core
        self.NTILES = (self.NC_NODES + 127) // 128       # 128-node tiles/core
        self.NC_PAD = self.NTILES * 128                  # padded nodes/core
        self.TROWS = self.NCORES * self.NC_PAD           # table rows (padded)
        self.N_WIN = (self.NC_NODES + self.WINW - 1) // self.WINW
        assert self.WINW == 128 and self.H == 128


# ---------------------------------------------------------------- host planner
def _table_row(cfg, node):
    """Global node id -> row in the AllGathered table.

    Table layout: rank-major, then partition-major within a core's [128, NTILES]
    z staging tile: row = (c*128 + p)*NTILES + t  with node = c*NC + t*128 + p."""
    c = node // cfg.NC_NODES
    local = node % cfg.NC_NODES
    t = local // 128
    p = local % 128
    return (c * 128 + p) * cfg.NTILES + t


def build_plan(cfg, edge_index, edge_weight):
    """Shared schedule + per-core staged arrays."""
    src = np.asarray(edge_index[0], np.int64)
    dst = np.asarray(edge_index[1], np.int64)
    ew = np.asarray(edge_weight, np.float32)

    loop = np.arange(cfg.N, dtype=np.int64)
    src = np.concatenate([src, loop])
    dst = np.concatenate([dst, loop])
    ew = np.concatenate([ew, np.ones(cfg.N, np.float32)])

    deg = np.zeros(cfg.N, np.float32)
    np.add.at(deg, dst, ew)
    dinv = (1.0 / np.sqrt(deg)).astype(np.float32)
    wnorm = (ew * dinv[src] * dinv[dst]).astype(np.float32)

    row = _table_row(cfg, src)
    seg = row // cfg.SEG
    n_seg = int(seg.max()) + 1
    assert n_seg <= 2 and cfg.TROWS - cfg.SEG < cfg.SEG
    n_seg = 2 if cfg.TROWS > cfg.SEG else 1

    core = dst // cfg.NC_NODES
    win = (dst % cfg.NC_NODES) // cfg.WINW

    order = np.lexsort((row, win, seg, core))
    src, dst, wnorm, core, win, seg, row = (
        a[order] for a in (src, dst, wnorm, core, win, seg, row))
    key = (core * n_seg + seg) * cfg.N_WIN + win
    starts = np.concatenate([[0], np.flatnonzero(np.diff(key)) + 1])
    ends = np.concatenate([starts[1:], [len(key)]])
    groups = {int(key[a]): (int(a), int(b)) for a, b in zip(starts, ends)}

    # shared chunk counts per (seg, win): max over cores, >= 1
    chunks_sw = np.zeros((n_seg, cfg.N_WIN), np.int64)
    for s in range(n_seg):
        for w in range(cfg.N_WIN):
            mx = max(
                groups.get((c * n_seg + s) * cfg.N_WIN + w, (0, 0))[1]
                - groups.get((c * n_seg + s) * cfg.N_WIN + w, (0, 0))[0]
                for c in range(cfg.NCORES))
            chunks_sw[s, w] = max(1, -(-mx // 128))
    total_chunks = int(chunks_sw.sum())

    # shared schedule: per chunk (seg, win, start, stop); batch spans per seg
    sched = []
    for s in range(n_seg):
        for w in range(cfg.N_WIN):
            for j in range(chunks_sw[s, w]):
                sched.append((s, w, j == 0, j == int(chunks_sw[s, w]) - 1))
    batches = []  # (chunk_lo, chunk_hi)
    pos = 0
    for s in range(n_seg):
        nch = int(chunks_sw[s].sum())
        for lo in range(0, nch, cfg.BATCH):
            batches.append((pos + lo, pos + min(lo + cfg.BATCH, nch)))
        pos += nch

    # per-core staged edge arrays
    per_core = []
    for c in range(cfg.NCORES):
        idx16 = np.zeros((total_chunks, 128), np.int16)
        drel = np.zeros((total_chunks, 128), np.float32)
        wn = np.zeros((total_chunks, 128), np.float32)
        pos = 0
        for s in range(n_seg):
            for w in range(cfg.N_WIN):
                k = (c * n_seg + s) * cfg.N_WIN + w
                nch = int(chunks_sw[s, w])
                if k in groups:
                    a, b = groups[k]
                    e_row = row[a:b] - s * cfg.SEG
                    e_rel = (dst[a:b] % cfg.NC_NODES) % cfg.WINW
                    e_wn = wnorm[a:b]
                else:
                    e_row = e_rel = np.zeros(0, np.int64)
                    e_wn = np.zeros(0, np.float32)
                pad = nch * 128 - len(e_row)
                idx16[pos:pos + nch] = np.concatenate(
                    [e_row, np.zeros(pad, np.int64)]).reshape(nch, 128)
                drel[pos:pos + nch] = np.concatenate(
                    [e_rel, np.zeros(pad, np.int64)]).reshape(nch, 128)
                wn[pos:pos + nch] = np.concatenate(
                    [e_wn, np.zeros(pad, np.float32)]).reshape(nch, 128)
                pos += nch

        # wrap indices for dma_gather: per batch, [p, s16] = idx[s16*16 + p%16]
        idx_wrap = np.zeros((128, total_chunks * 8), np.int16)
        for lo, hi in batches:
            flat = idx16[lo:hi].reshape(-1)            # (hi-lo)*128
            wrapped = flat.reshape(-1, 16).T           # [16, (hi-lo)*8]
            idx_wrap[:, lo * 8:hi * 8] = np.tile(wrapped, (8, 1))

        per_core.append(dict(
            idx_wrap=idx_wrap,
            dstrel=np.ascontiguousarray(drel.T),       # [128, total_chunks]
            wnorm=np.ascontiguousarray(wn.T),          # [128, total_chunks]
        ))

    plan = dict(n_seg=n_seg, chunks_sw=chunks_sw, total_chunks=total_chunks,
                sched=sched, batches=batches, dinv=dinv)
    return plan, per_core


def stage_core_inputs(cfg, plan, per_core, x, Ws, bs, fc_w, fc_b):
    """Build per-core in_maps for run_bass_kernel_spmd."""
    F, H, C = cfg.F, cfg.H, cfg.C
    KF = F // 128
    iota = np.tile(np.arange(cfg.WINW, dtype=np.float32), (128, 1))
    w1 = np.ascontiguousarray(
        np.asarray(Ws[0], np.float32).reshape(KF, 128, H))
    in_maps = []
    for c in range(cfg.NCORES):
        lo = c * cfg.NC_NODES
        xcol = np.zeros((F, cfg.NC_PAD), np.float32)
        xcol[:, :cfg.NC_NODES] = np.asarray(
            x[lo:lo + cfg.NC_NODES], np.float32).T
        xs = np.ascontiguousarray(xcol.reshape(KF, 128, cfg.NC_PAD))
        pc = per_core[c]
        in_maps.append({
            "x_in": xs,
            "idx_in": pc["idx_wrap"],
            "dstrel_in": pc["dstrel"],
            "wnorm_in": pc["wnorm"],
            "iota_in": iota,
            "w1_in": w1,
            "w2_in": np.asarray(Ws[1], np.float32),
            "w3_in": np.asarray(Ws[2], np.float32),
            "b1_in": np.asarray(bs[0], np.float32).reshape(H, 1),
            "b2_in": np.asarray(bs[1], np.float32).reshape(H, 1),
            "b3_in": np.asarray(bs[2], np.float32).reshape(H, 1),
            "fcw_in": np.asarray(fc_w, np.float32),
            "fcb_in": np.tile(np.asarray(fc_b, np.float32), (128, 1)),
        })
    return in_maps


# ---------------------------------------------------------------- device build
def build_nc(cfg, plan):
    f32 = mybir.dt.float32
    bf16 = mybir.dt.bfloat16
    i16 = mybir.dt.int16
    NT, NC_PAD, WINW, H, C = cfg.NTILES, cfg.NC_PAD, cfg.WINW, cfg.H, cfg.C
    KF = cfg.F // 128
    TC = plan["total_chunks"]
    n_seg = plan["n_seg"]
    sched, batches = plan["sched"], plan["batches"]

    nc = bacc.Bacc("TRN2", target_bir_lowering=False, debug=False,
                   num_devices=cfg.NCORES)
    dt_in = {}
    for name, shape, dt in [
        ("x_in", [KF, 128, NC_PAD], f32),
        ("idx_in", [128, TC * 8], i16),
        ("dstrel_in", [128, TC], f32),
        ("wnorm_in", [128, TC], f32),
        ("iota_in", [128, WINW], f32),
        ("w1_in", [KF, 128, H], f32),
        ("w2_in", [H, H], f32),
        ("w3_in", [H, H], f32),
        ("b1_in", [H, 1], f32),
        ("b2_in", [H, 1], f32),
        ("b3_in", [H, 1], f32),
        ("fcw_in", [H, C], f32),
        ("fcb_in", [128, C], f32),
    ]:
        dt_in[name] = nc.dram_tensor(name, shape, dt, kind="ExternalInput").ap()
    y_ap = nc.dram_tensor("y", [128, NT * C], f32, kind="ExternalOutput").ap()

    with tile.TileContext(nc) as tc, ExitStack() as ctx:
        P = lambda name, bufs=1, **kw: ctx.enter_context(
            tc.tile_pool(name=name, bufs=bufs, **kw))
        const_p = P("const")
        hp = P("h", bufs=2)
        zp = P("zsb", bufs=1)
        partial_p = P("partial")
        msgs_p = P("msgs", bufs=2)
        oh_p = P("onehot", bufs=6)
        psum_z = P("psum_z", bufs=2, space="PSUM")
        psum_agg = P("psum_agg", bufs=2, space="PSUM")
        psum_fc = P("psum_fc", bufs=2, space="PSUM")
        dram_p = P("dram", bufs=1, space="DRAM")
        sm_p = P("softmax", bufs=2)

        # ---- load constants / edge metadata into SBUF (once)
        def load(name, shape2d, dt, src_ap=None):
            t = const_p.tile(shape2d, dt, name=f"sb_{name}")
            nc.sync.dma_start(t[:], (src_ap if src_ap is not None
                                     else dt_in[name])[:])
            return t

        x_ctx = ExitStack()
        x_p = x_ctx.enter_context(tc.tile_pool(name="xpool", bufs=1))
        x_sb = x_p.tile([128, KF * NC_PAD], f32, name="x_sb")
        nc.sync.dma_start(
            x_sb[:].rearrange("p (k n) -> p k n", k=KF),
            dt_in["x_in"].rearrange("k p n -> p k n"))
        idx_sb = load("idx_in", [128, TC * 8], i16)
        drel_sb = load("dstrel_in", [128, TC], f32)
        wn_sb = load("wnorm_in", [128, TC], f32)
        iota_f = load("iota_in", [128, WINW], f32)
        iota_sb = const_p.tile([128, WINW], bf16, name="iota_bf")
        nc.vector.tensor_copy(iota_sb[:], iota_f[:])
        w1_sb = const_p.tile([128, KF * H], f32, name="w1_sb")
        nc.sync.dma_start(w1_sb[:].rearrange("p (k h) -> p k h", k=KF),
                          dt_in["w1_in"].rearrange("k p h -> p k h"))
        w2_sb = load("w2_in", [H, H], f32)
        w3_sb = load("w3_in", [H, H], f32)
        b_sb = [load(n, [H, 1], f32) for n in ("b1_in", "b2_in", "b3_in")]
        fcw_sb = load("fcw_in", [H, C], f32)
        fcb_sb = load("fcb_in", [128, C], f32)

        partial = partial_p.tile([128, cfg.N_WIN * WINW], f32, name="partial")

        ag_ins = [dram_p.tile([128, NT * H], bf16, name=f"ag_in{i}",
                              tag=f"ag_in{i}") for i in range(3)]
        ag_outs = [dram_p.tile([cfg.NCORES * 128, NT * H], bf16,
                               name=f"ag_out{i}", tag=f"ag_out{i}",
                               addr_space="Shared") for i in range(3)]

        h_cur = x_sb  # layer-1 input, [128, KF*NC_PAD]
        k_cur = KF
        for li in range(3):
            w_sb = (w1_sb, w2_sb, w3_sb)[li]
            # ---- dense: z[t] = h(:,t-slice).T @ W  (node-major psum), cast bf16
            z_sb = zp.tile([128, NT * H], bf16, name=f"z{li}", tag="z")
            for t in range(NT):
                pz = psum_z.tile([128, H], f32, tag="pz")
                for k in range(k_cur):
                    nc.tensor.matmul(
                        pz[:],
                        h_cur[:, k * NC_PAD + t * 128:
                              k * NC_PAD + (t + 1) * 128],
                        w_sb[:, k * H:(k + 1) * H] if k_cur > 1
                        else w_sb[:, :],
                        start=(k == 0), stop=(k == k_cur - 1))
                nc.scalar.copy(z_sb[:, t * H:(t + 1) * H], pz[:])
            if li == 0:
                x_ctx.close()  # free x_sb SBUF after layer-1 transform
            ag_in, ag_out = ag_ins[li], ag_outs[li]
            # gather-table view: row r=(c*128+p)*NT+t -> 128 bf16 els
            table = ag_out[:].rearrange("a (t h) -> (a t) h", h=H)
            nc.sync.dma_start(ag_in[:], z_sb[:])
            nc.gpsimd.collective_compute(
                "AllGather", mybir.AluOpType.bypass,
                replica_groups=[list(range(cfg.NCORES))],
                ins=[ag_in.opt()], outs=[ag_out.opt()])

            # ---- gather batches
            msgs_tiles = {}
            for bi, (lo, hi) in enumerate(batches):
                bc = hi - lo
                m = msgs_p.tile([128, cfg.BATCH, H], bf16, tag="msgs",
                                name=f"m{li}_{bi}")
                seg = sched[lo][0]
                seg_lo = seg * cfg.SEG
                seg_rows = min(cfg.SEG, cfg.TROWS - seg_lo)
                nc.gpsimd.dma_gather(
                    m[:, :bc, :],
                    table[seg_lo:seg_lo + seg_rows, :],
                    idx_sb[:, lo * 8:hi * 8],
                    bc * 128, bc * 128, H,
                    single_packet=cfg.SINGLE_PACKET)
                msgs_tiles[lo] = m

            # ---- scatter: weighted one-hot matmul, accumulate per window
            h_next = hp.tile([128, NC_PAD], f32, name=f"h{li + 1}", tag="h")
            pa = None
            for ci, (s, w, st, sp) in enumerate(sched):
                blo = max(l for (l, h_) in batches if l <= ci)
                m = msgs_tiles[blo]
                oh = oh_p.tile([128, WINW], bf16, tag="oh")
                nc.vector.tensor_scalar(
                    oh[:], iota_sb[:],
                    drel_sb[:, ci:ci + 1], wn_sb[:, ci:ci + 1],
                    mybir.AluOpType.is_equal, mybir.AluOpType.mult)
                if st:
                    pa = psum_agg.tile([128, WINW], f32, tag="pagg")
                nc.tensor.matmul(pa[:], m[:, ci - blo, :], oh[:],
                                 start=st, stop=sp)
                if sp:
                    wsl = slice(w * WINW, (w + 1) * WINW)
                    if s == 0 and n_seg == 2:
                        nc.scalar.copy(partial[:, wsl], pa[:])
                    elif n_seg == 2:
                        nc.vector.tensor_tensor(
                            h_next[:, wsl], pa[:], partial[:, wsl],
                            mybir.AluOpType.add)
                        nc.scalar.activation(
                            h_next[:, wsl], h_next[:, wsl],
                            mybir.ActivationFunctionType.Relu,
                            bias=b_sb[li][:])
                    else:
                        nc.scalar.activation(
                            h_next[:, wsl], pa[:],
                            mybir.ActivationFunctionType.Relu,
                            bias=b_sb[li][:])
            h_cur = h_next
            k_cur = 1

        # ---- fc head + softmax (node-major tiles)
        logit = sm_p.tile([128, NT, C], f32, name="logit")
        for t in range(NT):
            pf = psum_fc.tile([128, C], f32, tag="pfc")
            nc.tensor.matmul(pf[:], h_cur[:, t * 128:(t + 1) * 128],
                             fcw_sb[:], start=True, stop=True)
            nc.vector.tensor_tensor(logit[:, t, :], pf[:], fcb_sb[:],
                                    mybir.AluOpType.add)
        rmax = sm_p.tile([128, NT], f32, name="rmax")
        nc.vector.tensor_reduce(rmax[:], logit[:], mybir.AxisListType.X,
                                mybir.AluOpType.max)
        shifted = sm_p.tile([128, NT, C], f32, name="shifted")
        for t in range(NT):
            nc.vector.tensor_scalar(
                shifted[:, t, :], logit[:, t, :], rmax[:, t:t + 1], None,
                mybir.AluOpType.subtract)
        expd = sm_p.tile([128, NT, C], f32, name="expd")
        nc.scalar.activation(expd[:], shifted[:],
                             mybir.ActivationFunctionType.Exp)
        esum = sm_p.tile([128, NT], f32, name="esum")
        nc.vector.tensor_reduce(esum[:], expd[:], mybir.AxisListType.X,
                                mybir.AluOpType.add)
        rinv = sm_p.tile([128, NT], f32, name="rinv")
        nc.vector.reciprocal(rinv[:], esum[:])
        prob = sm_p.tile([128, NT, C], f32, name="prob")
        for t in range(NT):
            nc.vector.tensor_scalar(
                prob[:, t, :], expd[:, t, :], rinv[:, t:t + 1], None,
                mybir.AluOpType.mult)
        nc.sync.dma_start(y_ap[:], prob[:].rearrange("p t c -> p (t c)"))

    nc.compile()
    return nc


# ---------------------------------------------------------------- entry point
_CACHE = {}


def _get_built(cfg_key, cfg, edge_index, edge_weight):
    key = (cfg_key, hash(edge_index.tobytes()) ^ hash(edge_weight.tobytes()))
    if key not in _CACHE:
        plan, per_core = build_plan(cfg, edge_index, edge_weight)
        nc = build_nc(cfg, plan)
        _CACHE.clear()
        _CACHE[key] = (plan, per_core, nc)
    return _CACHE[key]


def kernel(x, edge_index, edge_weight, W1, b1, W2, b2, W3, b3, fc_w, fc_b,
           **extra):
    cfg = CFG()
    x = np.asarray(x, np.float32)
    edge_index = np.asarray(edge_index)
    edge_weight = np.asarray(edge_weight, np.float32)
    plan, per_core, nc = _get_built("default", cfg, edge_index, edge_weight)
    in_maps = stage_core_inputs(
        cfg, plan, per_core, x, [W1, W2, W3], [b1, b2, b3], fc_w, fc_b)
    res = run_bass_kernel_spmd(nc, in_maps, core_ids=list(range(cfg.NCORES)))
    out = np.zeros((cfg.N, cfg.C), np.float32)
    for c in range(cfg.NCORES):
        yc = res.results[c]["y"].reshape(128, cfg.NTILES, cfg.C)
        yc = yc.transpose(1, 0, 2).reshape(cfg.NC_PAD, cfg.C)
        out[c * cfg.NC_NODES:(c + 1) * cfg.NC_NODES] = yc[:cfg.NC_NODES]
    return out


# revision 20
# speedup vs baseline: 3711.6591x; 1.0063x over previous
"""Trainium2 Bass kernel for a 3-layer GCN (+linear head, softmax).

Contract: kernel(**inputs) takes FULL unsharded inputs (as produced by the
problem's setup_inputs()) and returns the FULL [50000, 10] float32 output.

Strategy (8 NeuronCores, graph/data parallel):
  - dst-node ranges sharded across cores (6250 nodes/core); small weight
    matrices replicated; self-loops appended to the edge list.
  - per GCN layer: each core computes z = h @ W for its own nodes (PE, f32),
    casts to bf16 and AllGathers the full 50k-row feature table into HBM.
  - per-edge source rows are fetched with gpsimd.dma_gather (256B rows);
    aggregation is a PE matmul against a weighted one-hot built on DVE with a
    single dual-op tensor_scalar (is_equal then mult); PSUM accumulates in f32.
  - all GCN normalization (dinv[src]*ew*dinv[dst]) is folded into the one-hot
    weights on the host (static graph preprocessing), so the device epilogue is
    just +bias and ReLU.
  - final linear head + softmax computed per node tile on device.

dma_gather's indices are int16, so the 50176-row table is split into two row
segments; each (window, segment) edge group is padded to 128-edge chunks and
the two segments are processed in two passes (pass-0 partial sums parked in
SBUF).
"""

import sys

sys.path.insert(0, "/opt/trn_rl_repo")

from contextlib import ExitStack

import numpy as np

import concourse.bass as bass
import concourse.tile as tile
from concourse import bacc, mybir
from concourse.bass_utils import run_bass_kernel_spmd


# ---------------------------------------------------------------- problem cfg
class CFG:
    N = 50000
    E = 800000
    F = 256
    H = 128
    C = 10
    NCORES = 8
    SEG = 32768          # int16 row-index limit for dma_gather
    WINW = 128           # dst window width (one-hot width, PSUM agg columns)
    BATCH = 32           # chunks (of 128 edges) per dma_gather call
    SINGLE_PACKET = False  # single_packet caps at 1024 idxs (64 desc/engine)

    def __init__(self, **kw):
        for k, v in kw.items():
            setattr(self, k, v)
        assert self.N % self.NCORES == 0
        self.NC_NODES = self.N // self.NCORES            # nodes per core
        self.NTILES = (self.NC_NODES + 127) // 128       # 128-node tiles/core
        self.NC_PAD = self.NTILES * 128                  # padded nodes/core
        self.TROWS = self.NCORES * self.NC_PAD           # table rows (padded)
        self.N_WIN = (self.NC_NODES + self.WINW - 1) // self.WINW
        assert self.WINW == 128 and self.H == 128


# ---------------------------------------------------------------- host planner
def _table_row(cfg, node):
    """Global node id -> row in the AllGathered table.

    Table layout: rank-major, then partition-major within a core's [128, NTILES]
    z staging tile: row = (c*128 + p)*NTILES + t  with node = c*NC + t*128 + p."""
    c = node // cfg.NC_NODES
    local = node % cfg.NC_NODES
    t = local // 128
    p = local % 128
    return (c * 128 + p) * cfg.NTILES + t


def build_plan(cfg, edge_index, edge_weight):
    """Shared schedule + per-core staged arrays."""
    src = np.asarray(edge_index[0], np.int64)
    dst = np.asarray(edge_index[1], np.int64)
    ew = np.asarray(edge_weight, np.float32)

    loop = np.arange(cfg.N, dtype=np.int64)
    src = np.concatenate([src, loop])
    dst = np.concatenate([dst, loop])
    ew = np.concatenate([ew, np.ones(cfg.N, np.float32)])

    deg = np.zeros(cfg.N, np.float32)
    np.add.at(deg, dst, ew)
    dinv = (1.0 / np.sqrt(deg)).astype(np.float32)
    wnorm = (ew * dinv[src] * dinv[dst]).astype(np.float32)

    row = _table_row(cfg, src)
    seg = row // cfg.SEG
    n_seg = int(seg.max()) + 1
    assert n_seg <= 2 and cfg.TROWS - cfg.SEG < cfg.SEG
    n_seg = 2 if cfg.TROWS > cfg.SEG else 1

    core = dst // cfg.NC_NODES
    win = (dst % cfg.NC_NODES) // cfg.WINW

    order = np.lexsort((row, win, seg, core))
    src, dst, wnorm, core, win, seg, row = (
        a[order] for a in (src, dst, wnorm, core, win, seg, row))
    key = (core * n_seg + seg) * cfg.N_WIN + win
    starts = np.concatenate([[0], np.flatnonzero(np.diff(key)) + 1])
    ends = np.concatenate([starts[1:], [len(key)]])
    groups = {int(key[a]): (int(a), int(b)) for a, b in zip(starts, ends)}

    # shared chunk counts per (seg, win): max over cores, >= 1
    chunks_sw = np.zeros((n_seg, cfg.N_WIN), np.int64)
    for s in range(n_seg):
        for w in range(cfg.N_WIN):
            mx = max(
                groups.get((c * n_seg + s) * cfg.N_WIN + w, (0, 0))[1]
                - groups.get((c * n_seg + s) * cfg.N_WIN + w, (0, 0))[0]
                for c in range(cfg.NCORES))
            chunks_sw[s, w] = max(1, -(-mx // 128))
    total_chunks = int(chunks_sw.sum())

    # shared schedule: per chunk (seg, win, start, stop); batch spans per seg
    sched = []
    for s in range(n_seg):
        for w in range(cfg.N_WIN):
            for j in range(chunks_sw[s, w]):
                sched.append((s, w, j == 0, j == int(chunks_sw[s, w]) - 1))
    batches = []  # (chunk_lo, chunk_hi)
    pos = 0
    for s in range(n_seg):
        nch = int(chunks_sw[s].sum())
        for lo in range(0, nch, cfg.BATCH):
            batches.append((pos + lo, pos + min(lo + cfg.BATCH, nch)))
        pos += nch

    # per-core staged edge arrays
    per_core = []
    for c in range(cfg.NCORES):
        idx16 = np.zeros((total_chunks, 128), np.int16)
        drel = np.zeros((total_chunks, 128), np.float32)
        wn = np.zeros((total_chunks, 128), np.float32)
        pos = 0
        for s in range(n_seg):
            for w in range(cfg.N_WIN):
                k = (c * n_seg + s) * cfg.N_WIN + w
                nch = int(chunks_sw[s, w])
                if k in groups:
                    a, b = groups[k]
                    e_row = row[a:b] - s * cfg.SEG
                    e_rel = (dst[a:b] % cfg.NC_NODES) % cfg.WINW
                    e_wn = wnorm[a:b]
                else:
                    e_row = e_rel = np.zeros(0, np.int64)
                    e_wn = np.zeros(0, np.float32)
                pad = nch * 128 - len(e_row)
                idx16[pos:pos + nch] = np.concatenate(
                    [e_row, np.zeros(pad, np.int64)]).reshape(nch, 128)
                drel[pos:pos + nch] = np.concatenate(
                    [e_rel, np.zeros(pad, np.int64)]).reshape(nch, 128)
                wn[pos:pos + nch] = np.concatenate(
                    [e_wn, np.zeros(pad, np.float32)]).reshape(nch, 128)
                pos += nch

        # wrap indices for dma_gather: per batch, [p, s16] = idx[s16*16 + p%16]
        idx_wrap = np.zeros((128, total_chunks * 8), np.int16)
        for lo, hi in batches:
            flat = idx16[lo:hi].reshape(-1)            # (hi-lo)*128
            wrapped = flat.reshape(-1, 16).T           # [16, (hi-lo)*8]
            idx_wrap[:, lo * 8:hi * 8] = np.tile(wrapped, (8, 1))

        per_core.append(dict(
            idx_wrap=idx_wrap,
            dstrel=np.ascontiguousarray(drel.T),       # [128, total_chunks]
            wnorm=np.ascontiguousarray(wn.T),          # [128, total_chunks]
        ))

    plan = dict(n_seg=n_seg, chunks_sw=chunks_sw, total_chunks=total_chunks,
                sched=sched, batches=batches, dinv=dinv)
    return plan, per_core


def stage_core_inputs(cfg, plan, per_core, x, Ws, bs, fc_w, fc_b):
    """Build per-core in_maps for run_bass_kernel_spmd."""
    F, H, C = cfg.F, cfg.H, cfg.C
    KF = F // 128
    iota = np.tile(np.arange(cfg.WINW, dtype=np.float32), (128, 1))
    w1 = np.ascontiguousarray(
        np.asarray(Ws[0], np.float32).reshape(KF, 128, H))
    in_maps = []
    for c in range(cfg.NCORES):
        lo = c * cfg.NC_NODES
        xcol = np.zeros((F, cfg.NC_PAD), np.float32)
        xcol[:, :cfg.NC_NODES] = np.asarray(
            x[lo:lo + cfg.NC_NODES], np.float32).T
        xs = np.ascontiguousarray(xcol.reshape(KF, 128, cfg.NC_PAD))
        pc = per_core[c]
        in_maps.append({
            "x_in": xs,
            "idx_in": pc["idx_wrap"],
            "dstrel_in": pc["dstrel"],
            "wnorm_in": pc["wnorm"],
            "iota_in": iota,
            "w1_in": w1,
            "w2_in": np.asarray(Ws[1], np.float32),
            "w3_in": np.asarray(Ws[2], np.float32),
            "b1_in": np.asarray(bs[0], np.float32).reshape(H, 1),
            "b2_in": np.asarray(bs[1], np.float32).reshape(H, 1),
            "b3_in": np.asarray(bs[2], np.float32).reshape(H, 1),
            "fcw_in": np.asarray(fc_w, np.float32),
            "fcb_in": np.tile(np.asarray(fc_b, np.float32), (128, 1)),
        })
    return in_maps


# ---------------------------------------------------------------- device build
def build_nc(cfg, plan):
    f32 = mybir.dt.float32
    bf16 = mybir.dt.bfloat16
    i16 = mybir.dt.int16
    NT, NC_PAD, WINW, H, C = cfg.NTILES, cfg.NC_PAD, cfg.WINW, cfg.H, cfg.C
    KF = cfg.F // 128
    TC = plan["total_chunks"]
    n_seg = plan["n_seg"]
    sched, batches = plan["sched"], plan["batches"]

    nc = bacc.Bacc("TRN2", target_bir_lowering=False, debug=False,
                   num_devices=cfg.NCORES)
    dt_in = {}
    for name, shape, dt in [
        ("x_in", [KF, 128, NC_PAD], f32),
        ("idx_in", [128, TC * 8], i16),
        ("dstrel_in", [128, TC], f32),
        ("wnorm_in", [128, TC], f32),
        ("iota_in", [128, WINW], f32),
        ("w1_in", [KF, 128, H], f32),
        ("w2_in", [H, H], f32),
        ("w3_in", [H, H], f32),
        ("b1_in", [H, 1], f32),
        ("b2_in", [H, 1], f32),
        ("b3_in", [H, 1], f32),
        ("fcw_in", [H, C], f32),
        ("fcb_in", [128, C], f32),
    ]:
        dt_in[name] = nc.dram_tensor(name, shape, dt, kind="ExternalInput").ap()
    y_ap = nc.dram_tensor("y", [128, NT * C], f32, kind="ExternalOutput").ap()

    with tile.TileContext(nc) as tc, ExitStack() as ctx:
        P = lambda name, bufs=1, **kw: ctx.enter_context(
            tc.tile_pool(name=name, bufs=bufs, **kw))
        const_p = P("const")
        hp = P("h", bufs=2)
        zp = P("zsb", bufs=1)
        partial_p = P("partial")
        msgs_p = P("msgs", bufs=2)
        oh_p = P("onehot", bufs=12)
        psum_z = P("psum_z", bufs=2, space="PSUM")
        psum_agg = P("psum_agg", bufs=3, space="PSUM")
        psum_fc = P("psum_fc", bufs=2, space="PSUM")
        dram_p = P("dram", bufs=1, space="DRAM")
        sm_p = P("softmax", bufs=2)

        # ---- load constants / edge metadata into SBUF (once)
        def load(name, shape2d, dt, src_ap=None):
            t = const_p.tile(shape2d, dt, name=f"sb_{name}")
            nc.sync.dma_start(t[:], (src_ap if src_ap is not None
                                     else dt_in[name])[:])
            return t

        x_ctx = ExitStack()
        x_p = x_ctx.enter_context(tc.tile_pool(name="xpool", bufs=1))
        x_sb = x_p.tile([128, KF * NC_PAD], f32, name="x_sb")
        nc.sync.dma_start(
            x_sb[:].rearrange("p (k n) -> p k n", k=KF),
            dt_in["x_in"].rearrange("k p n -> p k n"))
        idx_sb = load("idx_in", [128, TC * 8], i16)
        drel_sb = load("dstrel_in", [128, TC], f32)
        wn_sb = load("wnorm_in", [128, TC], f32)
        iota_f = load("iota_in", [128, WINW], f32)
        iota_sb = const_p.tile([128, WINW], bf16, name="iota_bf")
        nc.vector.tensor_copy(iota_sb[:], iota_f[:])
        w1_sb = const_p.tile([128, KF * H], f32, name="w1_sb")
        nc.sync.dma_start(w1_sb[:].rearrange("p (k h) -> p k h", k=KF),
                          dt_in["w1_in"].rearrange("k p h -> p k h"))
        w2_sb = load("w2_in", [H, H], f32)
        w3_sb = load("w3_in", [H, H], f32)
        b_sb = [load(n, [H, 1], f32) for n in ("b1_in", "b2_in", "b3_in")]
        fcw_sb = load("fcw_in", [H, C], f32)
        fcb_sb = load("fcb_in", [128, C], f32)

        partial = partial_p.tile([128, cfg.N_WIN * WINW], f32, name="partial")

        ag_ins = [dram_p.tile([128, NT * H], bf16, name=f"ag_in{i}",
                              tag=f"ag_in{i}") for i in range(3)]
        ag_outs = [dram_p.tile([cfg.NCORES * 128, NT * H], bf16,
                               name=f"ag_out{i}", tag=f"ag_out{i}",
                               addr_space="Shared") for i in range(3)]

        h_cur = x_sb  # layer-1 input, [128, KF*NC_PAD]
        k_cur = KF
        for li in range(3):
            w_sb = (w1_sb, w2_sb, w3_sb)[li]
            # ---- dense: z[t] = h(:,t-slice).T @ W  (node-major psum), cast bf16
            z_sb = zp.tile([128, NT * H], bf16, name=f"z{li}", tag="z")
            for t in range(NT):
                pz = psum_z.tile([128, H], f32, tag="pz")
                for k in range(k_cur):
                    nc.tensor.matmul(
                        pz[:],
                        h_cur[:, k * NC_PAD + t * 128:
                              k * NC_PAD + (t + 1) * 128],
                        w_sb[:, k * H:(k + 1) * H] if k_cur > 1
                        else w_sb[:, :],
                        start=(k == 0), stop=(k == k_cur - 1))
                nc.scalar.copy(z_sb[:, t * H:(t + 1) * H], pz[:])
            if li == 0:
                x_ctx.close()  # free x_sb SBUF after layer-1 transform
            ag_in, ag_out = ag_ins[li], ag_outs[li]
            # gather-table view: row r=(c*128+p)*NT+t -> 128 bf16 els
            table = ag_out[:].rearrange("a (t h) -> (a t) h", h=H)
            nc.sync.dma_start(ag_in[:], z_sb[:])
            nc.gpsimd.collective_compute(
                "AllGather", mybir.AluOpType.bypass,
                replica_groups=[list(range(cfg.NCORES))],
                ins=[ag_in.opt()], outs=[ag_out.opt()])

            # ---- gather batches
            msgs_tiles = {}
            for bi, (lo, hi) in enumerate(batches):
                bc = hi - lo
                m = msgs_p.tile([128, cfg.BATCH, H], bf16, tag="msgs",
                                name=f"m{li}_{bi}")
                seg = sched[lo][0]
                seg_lo = seg * cfg.SEG
                seg_rows = min(cfg.SEG, cfg.TROWS - seg_lo)
                nc.gpsimd.dma_gather(
                    m[:, :bc, :],
                    table[seg_lo:seg_lo + seg_rows, :],
                    idx_sb[:, lo * 8:hi * 8],
                    bc * 128, bc * 128, H,
                    single_packet=cfg.SINGLE_PACKET)
                msgs_tiles[lo] = m

            # ---- scatter: weighted one-hot matmul, accumulate per window
            h_next = hp.tile([128, NC_PAD], f32, name=f"h{li + 1}", tag="h")
            pa = None
            for ci, (s, w, st, sp) in enumerate(sched):
                blo = max(l for (l, h_) in batches if l <= ci)
                m = msgs_tiles[blo]
                oh = oh_p.tile([128, WINW], bf16, tag="oh")
                nc.vector.tensor_scalar(
                    oh[:], iota_sb[:],
                    drel_sb[:, ci:ci + 1], wn_sb[:, ci:ci + 1],
                    mybir.AluOpType.is_equal, mybir.AluOpType.mult)
                if st:
                    pa = psum_agg.tile([128, WINW], f32, tag="pagg")
                nc.tensor.matmul(pa[:], m[:, ci - blo, :], oh[:],
                                 start=st, stop=sp)
                if sp:
                    wsl = slice(w * WINW, (w + 1) * WINW)
                    if s == 0 and n_seg == 2:
                        nc.scalar.copy(partial[:, wsl], pa[:])
                    elif n_seg == 2:
                        nc.vector.tensor_tensor(
                            h_next[:, wsl], pa[:], partial[:, wsl],
                            mybir.AluOpType.add)
                        nc.scalar.activation(
                            h_next[:, wsl], h_next[:, wsl],
                            mybir.ActivationFunctionType.Relu,
                            bias=b_sb[li][:])
                    else:
                        nc.scalar.activation(
                            h_next[:, wsl], pa[:],
                            mybir.ActivationFunctionType.Relu,
                            bias=b_sb[li][:])
            h_cur = h_next
            k_cur = 1

        # ---- fc head + softmax (node-major tiles)
        logit = sm_p.tile([128, NT, C], f32, name="logit")
        for t in range(NT):
            pf = psum_fc.tile([128, C], f32, tag="pfc")
            nc.tensor.matmul(pf[:], h_cur[:, t * 128:(t + 1) * 128],
                             fcw_sb[:], start=True, stop=True)
            nc.vector.tensor_tensor(logit[:, t, :], pf[:], fcb_sb[:],
                                    mybir.AluOpType.add)
        rmax = sm_p.tile([128, NT], f32, name="rmax")
        nc.vector.tensor_reduce(rmax[:], logit[:], mybir.AxisListType.X,
                                mybir.AluOpType.max)
        shifted = sm_p.tile([128, NT, C], f32, name="shifted")
        for t in range(NT):
            nc.vector.tensor_scalar(
                shifted[:, t, :], logit[:, t, :], rmax[:, t:t + 1], None,
                mybir.AluOpType.subtract)
        expd = sm_p.tile([128, NT, C], f32, name="expd")
        nc.scalar.activation(expd[:], shifted[:],
                             mybir.ActivationFunctionType.Exp)
        esum = sm_p.tile([128, NT], f32, name="esum")
        nc.vector.tensor_reduce(esum[:], expd[:], mybir.AxisListType.X,
                                mybir.AluOpType.add)
        rinv = sm_p.tile([128, NT], f32, name="rinv")
        nc.vector.reciprocal(rinv[:], esum[:])
        prob = sm_p.tile([128, NT, C], f32, name="prob")
        for t in range(NT):
            nc.vector.tensor_scalar(
                prob[:, t, :], expd[:, t, :], rinv[:, t:t + 1], None,
                mybir.AluOpType.mult)
        nc.sync.dma_start(y_ap[:], prob[:].rearrange("p t c -> p (t c)"))

    nc.compile()
    return nc


# ---------------------------------------------------------------- entry point
_CACHE = {}


def _get_built(cfg_key, cfg, edge_index, edge_weight):
    key = (cfg_key, hash(edge_index.tobytes()) ^ hash(edge_weight.tobytes()))
    if key not in _CACHE:
        plan, per_core = build_plan(cfg, edge_index, edge_weight)
        nc = build_nc(cfg, plan)
        _CACHE.clear()
        _CACHE[key] = (plan, per_core, nc)
    return _CACHE[key]


def kernel(x, edge_index, edge_weight, W1, b1, W2, b2, W3, b3, fc_w, fc_b,
           **extra):
    cfg = CFG()
    x = np.asarray(x, np.float32)
    edge_index = np.asarray(edge_index)
    edge_weight = np.asarray(edge_weight, np.float32)
    plan, per_core, nc = _get_built("default", cfg, edge_index, edge_weight)
    in_maps = stage_core_inputs(
        cfg, plan, per_core, x, [W1, W2, W3], [b1, b2, b3], fc_w, fc_b)
    res = run_bass_kernel_spmd(nc, in_maps, core_ids=list(range(cfg.NCORES)))
    out = np.zeros((cfg.N, cfg.C), np.float32)
    for c in range(cfg.NCORES):
        yc = res.results[c]["y"].reshape(128, cfg.NTILES, cfg.C)
        yc = yc.transpose(1, 0, 2).reshape(cfg.NC_PAD, cfg.C)
        out[c * cfg.NC_NODES:(c + 1) * cfg.NC_NODES] = yc[:cfg.NC_NODES]
    return out


# revision 32
# speedup vs baseline: 3953.9090x; 1.0653x over previous
"""Trainium2 Bass kernel for a 3-layer GCN (+linear head, softmax).

Contract: kernel(**inputs) takes FULL unsharded inputs (as produced by the
problem's setup_inputs()) and returns the FULL [50000, 10] float32 output.

Strategy (8 NeuronCores, graph/data parallel):
  - dst-node ranges sharded across cores (6250 nodes/core); small weight
    matrices replicated; self-loops appended to the edge list.
  - per GCN layer: each core computes z = h @ W for its own nodes (PE, f32),
    casts to bf16 and AllGathers the full 50k-row feature table into HBM.
  - per-edge source rows are fetched with gpsimd.dma_gather (256B rows);
    aggregation is a PE matmul against a weighted one-hot built on DVE with a
    single dual-op tensor_scalar (is_equal then mult); PSUM accumulates in f32.
  - all GCN normalization (dinv[src]*ew*dinv[dst]) is folded into the one-hot
    weights on the host (static graph preprocessing), so the device epilogue is
    just +bias and ReLU.
  - final linear head + softmax computed per node tile on device.

dma_gather's indices are int16, so the feature table is built as TWO tables
(node tiles [0, NT_A) and [NT_A, NT)), each AllGathered separately; each
(window, table) edge group is padded to 128-edge chunks and the two tables are
processed in two passes (pass-0 partial sums parked in SBUF). The split also
pipelines the layer boundary: AG_a fires halfway through the z phase and
pass-0 gathers only wait on it, while AG_b overlaps with aggregation.
"""

import sys

sys.path.insert(0, "/opt/trn_rl_repo")

from contextlib import ExitStack

import numpy as np

import concourse.bass as bass
import concourse.tile as tile
from concourse import bacc, mybir
from concourse.bass_utils import run_bass_kernel_spmd


# ---------------------------------------------------------------- problem cfg
class CFG:
    N = 50000
    E = 800000
    F = 256
    H = 128
    C = 10
    NCORES = 8
    SEG = 32768          # int16 row-index limit for dma_gather
    WINW = 128           # dst window width (one-hot width, PSUM agg columns)
    BATCH = 32           # chunks (of 128 edges) per dma_gather call
    SINGLE_PACKET = False  # single_packet caps at 1024 idxs (64 desc/engine)

    def __init__(self, **kw):
        for k, v in kw.items():
            setattr(self, k, v)
        assert self.N % self.NCORES == 0
        self.NC_NODES = self.N // self.NCORES            # nodes per core
        self.NTILES = (self.NC_NODES + 127) // 128       # 128-node tiles/core
        self.NC_PAD = self.NTILES * 128                  # padded nodes/core
        self.N_WIN = (self.NC_NODES + self.WINW - 1) // self.WINW
        # two gather tables (AllGather halves), split by node-tile range
        assert self.NTILES >= 2
        self.NT_A = (self.NTILES + 1) // 2
        self.NT_B = self.NTILES - self.NT_A
        self.ROWS_A = self.NCORES * 128 * self.NT_A
        self.ROWS_B = self.NCORES * 128 * self.NT_B
        assert self.ROWS_A <= 32768 and self.ROWS_B <= 32768, \
            "int16 dma_gather index range"
        assert self.WINW == 128 and self.H == 128


# ---------------------------------------------------------------- host planner
def _table_row(cfg, node):
    """Global node id -> (table id, row) in the AllGathered half-tables.

    Table a holds node tiles [0, NT_A), table b the rest. Each table is the
    rank-concat of per-core [128, NT_x] z staging slices: row =
    (c*128 + p)*NT_x + (t - t0)  with node = c*NC + t*128 + p."""
    c = node // cfg.NC_NODES
    local = node % cfg.NC_NODES
    t = local // 128
    p = local % 128
    seg = np.where(t < cfg.NT_A, 0, 1)
    row = np.where(t < cfg.NT_A,
                   (c * 128 + p) * cfg.NT_A + t,
                   (c * 128 + p) * cfg.NT_B + (t - cfg.NT_A))
    return seg, row


def build_plan(cfg, edge_index, edge_weight):
    """Shared schedule + per-core staged arrays."""
    src = np.asarray(edge_index[0], np.int64)
    dst = np.asarray(edge_index[1], np.int64)
    ew = np.asarray(edge_weight, np.float32)

    loop = np.arange(cfg.N, dtype=np.int64)
    src = np.concatenate([src, loop])
    dst = np.concatenate([dst, loop])
    ew = np.concatenate([ew, np.ones(cfg.N, np.float32)])

    deg = np.zeros(cfg.N, np.float32)
    np.add.at(deg, dst, ew)
    dinv = (1.0 / np.sqrt(deg)).astype(np.float32)
    wnorm = (ew * dinv[src] * dinv[dst]).astype(np.float32)

    seg, row = _table_row(cfg, src)
    n_seg = 2

    core = dst // cfg.NC_NODES
    win = (dst % cfg.NC_NODES) // cfg.WINW

    order = np.lexsort((row, win, seg, core))
    src, dst, wnorm, core, win, seg, row = (
        a[order] for a in (src, dst, wnorm, core, win, seg, row))
    key = (core * n_seg + seg) * cfg.N_WIN + win
    starts = np.concatenate([[0], np.flatnonzero(np.diff(key)) + 1])
    ends = np.concatenate([starts[1:], [len(key)]])
    groups = {int(key[a]): (int(a), int(b)) for a, b in zip(starts, ends)}

    # shared chunk counts per (seg, win): max over cores, >= 1
    chunks_sw = np.zeros((n_seg, cfg.N_WIN), np.int64)
    for s in range(n_seg):
        for w in range(cfg.N_WIN):
            mx = max(
                groups.get((c * n_seg + s) * cfg.N_WIN + w, (0, 0))[1]
                - groups.get((c * n_seg + s) * cfg.N_WIN + w, (0, 0))[0]
                for c in range(cfg.NCORES))
            chunks_sw[s, w] = max(1, -(-mx // 128))
    total_chunks = int(chunks_sw.sum())

    # shared schedule: per chunk (seg, win, start, stop); batch spans per seg
    sched = []
    for s in range(n_seg):
        for w in range(cfg.N_WIN):
            for j in range(chunks_sw[s, w]):
                sched.append((s, w, j == 0, j == int(chunks_sw[s, w]) - 1))
    batches = []  # (chunk_lo, chunk_hi)
    pos = 0
    for s in range(n_seg):
        nch = int(chunks_sw[s].sum())
        for lo in range(0, nch, cfg.BATCH):
            batches.append((pos + lo, pos + min(lo + cfg.BATCH, nch)))
        pos += nch

    # per-core staged edge arrays
    per_core = []
    for c in range(cfg.NCORES):
        idx16 = np.zeros((total_chunks, 128), np.int16)
        drel = np.zeros((total_chunks, 128), np.float32)
        wn = np.zeros((total_chunks, 128), np.float32)
        pos = 0
        for s in range(n_seg):
            for w in range(cfg.N_WIN):
                k = (c * n_seg + s) * cfg.N_WIN + w
                nch = int(chunks_sw[s, w])
                if k in groups:
                    a, b = groups[k]
                    e_row = row[a:b]
                    e_rel = (dst[a:b] % cfg.NC_NODES) % cfg.WINW
                    e_wn = wnorm[a:b]
                else:
                    e_row = e_rel = np.zeros(0, np.int64)
                    e_wn = np.zeros(0, np.float32)
                pad = nch * 128 - len(e_row)
                idx16[pos:pos + nch] = np.concatenate(
                    [e_row, np.zeros(pad, np.int64)]).reshape(nch, 128)
                drel[pos:pos + nch] = np.concatenate(
                    [e_rel, np.zeros(pad, np.int64)]).reshape(nch, 128)
                wn[pos:pos + nch] = np.concatenate(
                    [e_wn, np.zeros(pad, np.float32)]).reshape(nch, 128)
                pos += nch

        # wrap indices for dma_gather: per batch, [p, s16] = idx[s16*16 + p%16]
        idx_wrap = np.zeros((128, total_chunks * 8), np.int16)
        for lo, hi in batches:
            flat = idx16[lo:hi].reshape(-1)            # (hi-lo)*128
            wrapped = flat.reshape(-1, 16).T           # [16, (hi-lo)*8]
            idx_wrap[:, lo * 8:hi * 8] = np.tile(wrapped, (8, 1))

        per_core.append(dict(
            idx_wrap=idx_wrap,
            dstrel=np.ascontiguousarray(drel.T),       # [128, total_chunks]
            wnorm=np.ascontiguousarray(wn.T),          # [128, total_chunks]
        ))

    plan = dict(n_seg=n_seg, chunks_sw=chunks_sw, total_chunks=total_chunks,
                sched=sched, batches=batches, dinv=dinv)
    return plan, per_core


def stage_core_inputs(cfg, plan, per_core, x, Ws, bs, fc_w, fc_b):
    """Build per-core in_maps for run_bass_kernel_spmd."""
    F, H, C = cfg.F, cfg.H, cfg.C
    KF = F // 128
    iota = np.tile(np.arange(cfg.WINW, dtype=np.float32), (128, 1))
    w1 = np.ascontiguousarray(
        np.asarray(Ws[0], np.float32).reshape(KF, 128, H))
    in_maps = []
    for c in range(cfg.NCORES):
        lo = c * cfg.NC_NODES
        xcol = np.zeros((F, cfg.NC_PAD), np.float32)
        xcol[:, :cfg.NC_NODES] = np.asarray(
            x[lo:lo + cfg.NC_NODES], np.float32).T
        xs = np.ascontiguousarray(xcol.reshape(KF, 128, cfg.NC_PAD))
        pc = per_core[c]
        in_maps.append({
            "x_in": xs,
            "idx_in": pc["idx_wrap"],
            "dstrel_in": pc["dstrel"],
            "wnorm_in": pc["wnorm"],
            "iota_in": iota,
            "w1_in": w1,
            "w2_in": np.asarray(Ws[1], np.float32),
            "w3_in": np.asarray(Ws[2], np.float32),
            "b1_in": np.asarray(bs[0], np.float32).reshape(H, 1),
            "b2_in": np.asarray(bs[1], np.float32).reshape(H, 1),
            "b3_in": np.asarray(bs[2], np.float32).reshape(H, 1),
            "fcw_in": np.asarray(fc_w, np.float32),
            "fcb_in": np.tile(np.asarray(fc_b, np.float32), (128, 1)),
            "eye_in": np.eye(128, dtype=np.float32),
        })
    return in_maps


# ---------------------------------------------------------------- device build
def build_nc(cfg, plan):
    f32 = mybir.dt.float32
    bf16 = mybir.dt.bfloat16
    i16 = mybir.dt.int16
    NT, NC_PAD, WINW, H, C = cfg.NTILES, cfg.NC_PAD, cfg.WINW, cfg.H, cfg.C
    KF = cfg.F // 128
    TC = plan["total_chunks"]
    n_seg = plan["n_seg"]
    sched, batches = plan["sched"], plan["batches"]

    nc = bacc.Bacc("TRN2", target_bir_lowering=False, debug=False,
                   num_devices=cfg.NCORES)
    dt_in = {}
    for name, shape, dt in [
        ("x_in", [KF, 128, NC_PAD], f32),
        ("idx_in", [128, TC * 8], i16),
        ("dstrel_in", [128, TC], f32),
        ("wnorm_in", [128, TC], f32),
        ("iota_in", [128, WINW], f32),
        ("w1_in", [KF, 128, H], f32),
        ("w2_in", [H, H], f32),
        ("w3_in", [H, H], f32),
        ("b1_in", [H, 1], f32),
        ("b2_in", [H, 1], f32),
        ("b3_in", [H, 1], f32),
        ("fcw_in", [H, C], f32),
        ("fcb_in", [128, C], f32),
        ("eye_in", [128, 128], f32),
    ]:
        dt_in[name] = nc.dram_tensor(name, shape, dt, kind="ExternalInput").ap()
    y_ap = nc.dram_tensor("y", [128, NT * C], f32, kind="ExternalOutput").ap()

    with tile.TileContext(nc) as tc, ExitStack() as ctx:
        P = lambda name, bufs=1, **kw: ctx.enter_context(
            tc.tile_pool(name=name, bufs=bufs, **kw))
        const_p = P("const")
        hp = P("h", bufs=2)
        zp = P("zsb", bufs=1)
        partial_p = P("partial")
        msgs_p = P("msgs", bufs=2)
        oh_p = P("onehot", bufs=12)
        psum_z = P("psum_z", bufs=2, space="PSUM")
        psum_agg = P("psum_agg", bufs=3, space="PSUM")
        psum_fc = P("psum_fc", bufs=2, space="PSUM")
        dram_p = P("dram", bufs=1, space="DRAM")
        sm_p = P("softmax", bufs=2)

        # ---- load constants / edge metadata into SBUF (once)
        def load(name, shape2d, dt, src_ap=None):
            t = const_p.tile(shape2d, dt, name=f"sb_{name}")
            nc.sync.dma_start(t[:], (src_ap if src_ap is not None
                                     else dt_in[name])[:])
            return t

        x_ctx = ExitStack()
        x_p = x_ctx.enter_context(tc.tile_pool(name="xpool", bufs=1))
        x_sb = x_p.tile([128, KF * NC_PAD], f32, name="x_sb")
        nc.sync.dma_start(
            x_sb[:].rearrange("p (k n) -> p k n", k=KF),
            dt_in["x_in"].rearrange("k p n -> p k n"))
        idx_sb = load("idx_in", [128, TC * 8], i16)
        drel_sb = load("dstrel_in", [128, TC], f32)
        wn_sb = load("wnorm_in", [128, TC], f32)
        iota_f = load("iota_in", [128, WINW], f32)
        iota_sb = const_p.tile([128, WINW], bf16, name="iota_bf")
        nc.vector.tensor_copy(iota_sb[:], iota_f[:])
        w1_sb = const_p.tile([128, KF * H], f32, name="w1_sb")
        nc.sync.dma_start(w1_sb[:].rearrange("p (k h) -> p k h", k=KF),
                          dt_in["w1_in"].rearrange("k p h -> p k h"))
        w2_sb = load("w2_in", [H, H], f32)
        w3_sb = load("w3_in", [H, H], f32)
        b_sb = [load(n, [H, 1], f32) for n in ("b1_in", "b2_in", "b3_in")]
        fcw_sb = load("fcw_in", [H, C], f32)
        fcb_sb = load("fcb_in", [128, C], f32)
        eye_sb = load("eye_in", [128, 128], f32)

        partial = partial_p.tile([128, cfg.N_WIN * WINW], f32, name="partial")

        NT_A, NT_B = cfg.NT_A, cfg.NT_B
        ag_ins = [
            (dram_p.tile([128, NT_A * H], bf16, name=f"ag_ina{i}",
                         tag=f"ag_ina{i}"),
             dram_p.tile([128, NT_B * H], bf16, name=f"ag_inb{i}",
                         tag=f"ag_inb{i}")) for i in range(3)]
        ag_outs = [
            (dram_p.tile([cfg.NCORES * 128, NT_A * H], bf16,
                         name=f"ag_outa{i}", tag=f"ag_outa{i}",
                         addr_space="Shared"),
             dram_p.tile([cfg.NCORES * 128, NT_B * H], bf16,
                         name=f"ag_outb{i}", tag=f"ag_outb{i}",
                         addr_space="Shared")) for i in range(3)]

        h_cur = x_sb  # layer-1 input, [128, KF*NC_PAD]
        k_cur = KF
        for li in range(3):
            w_sb = (w1_sb, w2_sb, w3_sb)[li]
            # ---- dense: z[t] = h(:,t-slice).T @ W  (node-major psum), cast
            # bf16. Each half of the tiles feeds its own AllGather so AG_a
            # fires mid-phase and pass-0 gathers only wait on it.
            z_sb = zp.tile([128, NT * H], bf16, name=f"z{li}", tag="z")
            for t in range(NT):
                pz = psum_z.tile([128, H], f32, tag="pz")
                for k in range(k_cur):
                    nc.tensor.matmul(
                        pz[:],
                        h_cur[:, k * NC_PAD + t * 128:
                              k * NC_PAD + (t + 1) * 128],
                        w_sb[:, k * H:(k + 1) * H] if k_cur > 1
                        else w_sb[:, :],
                        start=(k == 0), stop=(k == k_cur - 1))
                nc.scalar.copy(z_sb[:, t * H:(t + 1) * H], pz[:])
                if t == NT_A - 1:
                    nc.sync.dma_start(ag_ins[li][0][:],
                                      z_sb[:, :NT_A * H])
                    nc.gpsimd.collective_compute(
                        "AllGather", mybir.AluOpType.bypass,
                        replica_groups=[list(range(cfg.NCORES))],
                        ins=[ag_ins[li][0].opt()],
                        outs=[ag_outs[li][0].opt()])
            if li == 0:
                x_ctx.close()  # free x_sb SBUF after layer-1 transform
            nc.sync.dma_start(ag_ins[li][1][:], z_sb[:, NT_A * H:])
            nc.gpsimd.collective_compute(
                "AllGather", mybir.AluOpType.bypass,
                replica_groups=[list(range(cfg.NCORES))],
                ins=[ag_ins[li][1].opt()], outs=[ag_outs[li][1].opt()])
            # gather-table views: row r=(c*128+p)*NT_x+(t-t0) -> 128 bf16 els
            tables = [
                ag_outs[li][0][:].rearrange("a (t h) -> (a t) h", h=H),
                ag_outs[li][1][:].rearrange("a (t h) -> (a t) h", h=H)]

            # ---- gather batches
            msgs_tiles = {}
            for bi, (lo, hi) in enumerate(batches):
                bc = hi - lo
                m = msgs_p.tile([128, cfg.BATCH, H], bf16, tag="msgs",
                                name=f"m{li}_{bi}")
                seg = sched[lo][0]
                nc.gpsimd.dma_gather(
                    m[:, :bc, :],
                    tables[seg][:, :],
                    idx_sb[:, lo * 8:hi * 8],
                    bc * 128, bc * 128, H,
                    single_packet=cfg.SINGLE_PACKET)
                msgs_tiles[lo] = m

            # ---- scatter: weighted one-hot matmul, accumulate per window
            h_next = hp.tile([128, NC_PAD], f32, name=f"h{li + 1}", tag="h")
            pa = None
            for ci, (s, w, st, sp) in enumerate(sched):
                blo = max(l for (l, h_) in batches if l <= ci)
                m = msgs_tiles[blo]
                oh = oh_p.tile([128, WINW], bf16, tag="oh")
                nc.vector.tensor_scalar(
                    oh[:], iota_sb[:],
                    drel_sb[:, ci:ci + 1], wn_sb[:, ci:ci + 1],
                    mybir.AluOpType.is_equal, mybir.AluOpType.mult)
                if st:
                    pa = psum_agg.tile([128, WINW], f32, tag="pagg")
                nc.tensor.matmul(pa[:], m[:, ci - blo, :], oh[:],
                                 start=st, stop=sp and s == 0)
                if sp:
                    wsl = slice(w * WINW, (w + 1) * WINW)
                    if s == 0:
                        nc.scalar.copy(partial[:, wsl], pa[:])
                    else:
                        # fold the pass-0 partial back in on PE (identity
                        # matmul closes the accumulation group), then Relu+b
                        # straight from PSUM on ACT -- no DVE work at all.
                        nc.tensor.matmul(pa[:], eye_sb[:], partial[:, wsl],
                                         start=False, stop=True)
                        nc.scalar.activation(
                            h_next[:, wsl], pa[:],
                            mybir.ActivationFunctionType.Relu,
                            bias=b_sb[li][:])
            h_cur = h_next
            k_cur = 1

        # ---- fc head + softmax (node-major tiles)
        logit = sm_p.tile([128, NT, C], f32, name="logit")
        for t in range(NT):
            pf = psum_fc.tile([128, C], f32, tag="pfc")
            nc.tensor.matmul(pf[:], h_cur[:, t * 128:(t + 1) * 128],
                             fcw_sb[:], start=True, stop=True)
            nc.vector.tensor_tensor(logit[:, t, :], pf[:], fcb_sb[:],
                                    mybir.AluOpType.add)
        rmax = sm_p.tile([128, NT], f32, name="rmax")
        nc.vector.tensor_reduce(rmax[:], logit[:], mybir.AxisListType.X,
                                mybir.AluOpType.max)
        shifted = sm_p.tile([128, NT, C], f32, name="shifted")
        for t in range(NT):
            nc.vector.tensor_scalar(
                shifted[:, t, :], logit[:, t, :], rmax[:, t:t + 1], None,
                mybir.AluOpType.subtract)
        expd = sm_p.tile([128, NT, C], f32, name="expd")
        nc.scalar.activation(expd[:], shifted[:],
                             mybir.ActivationFunctionType.Exp)
        esum = sm_p.tile([128, NT], f32, name="esum")
        nc.vector.tensor_reduce(esum[:], expd[:], mybir.AxisListType.X,
                                mybir.AluOpType.add)
        rinv = sm_p.tile([128, NT], f32, name="rinv")
        nc.vector.reciprocal(rinv[:], esum[:])
        prob = sm_p.tile([128, NT, C], f32, name="prob")
        for t in range(NT):
            nc.vector.tensor_scalar(
                prob[:, t, :], expd[:, t, :], rinv[:, t:t + 1], None,
                mybir.AluOpType.mult)
        nc.sync.dma_start(y_ap[:], prob[:].rearrange("p t c -> p (t c)"))

    nc.compile()
    return nc


# ---------------------------------------------------------------- entry point
_CACHE = {}


def _get_built(cfg_key, cfg, edge_index, edge_weight):
    key = (cfg_key, hash(edge_index.tobytes()) ^ hash(edge_weight.tobytes()))
    if key not in _CACHE:
        plan, per_core = build_plan(cfg, edge_index, edge_weight)
        nc = build_nc(cfg, plan)
        _CACHE.clear()
        _CACHE[key] = (plan, per_core, nc)
    return _CACHE[key]


def kernel(x, edge_index, edge_weight, W1, b1, W2, b2, W3, b3, fc_w, fc_b,
           **extra):
    cfg = CFG()
    x = np.asarray(x, np.float32)
    edge_index = np.asarray(edge_index)
    edge_weight = np.asarray(edge_weight, np.float32)
    plan, per_core, nc = _get_built("default", cfg, edge_index, edge_weight)
    in_maps = stage_core_inputs(
        cfg, plan, per_core, x, [W1, W2, W3], [b1, b2, b3], fc_w, fc_b)
    res = run_bass_kernel_spmd(nc, in_maps, core_ids=list(range(cfg.NCORES)))
    out = np.zeros((cfg.N, cfg.C), np.float32)
    for c in range(cfg.NCORES):
        yc = res.results[c]["y"].reshape(128, cfg.NTILES, cfg.C)
        yc = yc.transpose(1, 0, 2).reshape(cfg.NC_PAD, cfg.C)
        out[c * cfg.NC_NODES:(c + 1) * cfg.NC_NODES] = yc[:cfg.NC_NODES]
    return out
